# revision 9
# baseline (speedup 1.0000x reference)
"""LoRA-with-routing kernel for Trainium2 (8 NeuronCores, SPMD).

out[b] = base[b] + (x[b] @ lora_A[idx[b]]) @ lora_B[idx[b]] * s[idx[b]]

Sharding: data-parallel over batch (B=8 rows, one per core). The adapter
gather (routing) happens host-side while sharding: each core receives its
batch row plus that row's adapter weights (scale folded into B, cast bf16).
x is laid out [D, T] per core (transposed during sharding) so the GEMM1
contraction dim lands on SBUF partitions with unit-stride DMA.

Device pipeline per core (T=2048, D=4096, R=64), per 512-token group:
  1. SWDGE cast-load xT d-chunk f32->bf16      [128 d, 512 t]  x32
  2. GEMM1 (PE): interT[64 r, 512 t] += A_c.T @ xT_c  (accum 32 d-chunks)
  3. DVE evac interT -> bf16 SBUF
  4. per 128-token subtile: load base, GEMM2 y[128,512] = interT.T @ B,
     add into base (DVE/ACT), store f32
"""

import sys

for _p in ("/opt/trn_rl_repo", "/root/.axon_site/_ro/trn_rl_repo"):
    if _p not in sys.path:
        sys.path.append(_p)

import numpy as np
import ml_dtypes

import concourse.bass as bass
import concourse.bacc as bacc
import concourse.mybir as mybir
from concourse import tile

B, T, D, R = 8, 2048, 4096, 64
P = 128          # partitions
DC = D // P      # 32 d-chunks (contraction)
TG = 512         # token group (GEMM1 moving dim, one PSUM bank of f32)
OCH = 512        # output free chunk (one PSUM bank of f32)
OC = D // OCH    # 8 o-chunks

F32 = mybir.dt.float32
BF16 = mybir.dt.bfloat16


def build_program(t_tokens: int = T):
    ng = t_tokens // TG
    nc = bacc.Bacc("TRN2", target_bir_lowering=False, debug=False, num_devices=B)
    xt = nc.dram_tensor("xt", [D, t_tokens], F32, kind="ExternalInput").ap()
    base = nc.dram_tensor("base", [t_tokens, D], F32, kind="ExternalInput").ap()
    a_w = nc.dram_tensor("a_w", [D, R], BF16, kind="ExternalInput").ap()
    b_w = nc.dram_tensor("b_w", [R, D], BF16, kind="ExternalInput").ap()
    out = nc.dram_tensor("out", [t_tokens, D], F32, kind="ExternalOutput").ap()

    with tile.TileContext(nc) as tc:
        _body(tc, xt, base, a_w, b_w, out, ng)
    nc.compile()
    return nc


def _body(tc, xt, base, a_w, b_w, out, ng):
    nc = tc.nc
    with (
        tc.tile_pool(name="const", bufs=1) as cpool,
        tc.tile_pool(name="xc", bufs=8) as xc_pool,
        tc.tile_pool(name="bs", bufs=3) as bs_pool,
        tc.tile_pool(name="it", bufs=2) as it_pool,
        tc.tile_pool(name="ps1", bufs=2, space="PSUM") as ps1,
        tc.tile_pool(name="ps2", bufs=4, space="PSUM") as ps2,
    ):
        # Adapter weights, loaded once.
        # a_sb[p, c, r] = A[c*128 + p, r]  (contraction dim on partitions)
        a_sb = cpool.tile([P, DC, R], BF16)
        nc.sync.dma_start(a_sb[:], a_w.rearrange("(c p) r -> p c r", p=P))
        # b_sb[r, o] on partitions 0..63
        b_sb = cpool.tile([R, D], BF16)
        nc.sync.dma_start(b_sb[:], b_w[:])

        for g in range(ng):
            t0 = g * TG
            # GEMM1: interT[r, t] = sum_c A_c.T @ xT_c, accumulated in PSUM
            it_ps = ps1.tile([R, TG], F32)
            for c in range(DC):
                xc = xc_pool.tile([P, TG], BF16)
                nc.gpsimd.dma_start(
                    xc[:], xt[c * P : (c + 1) * P, t0 : t0 + TG]
                )  # SWDGE casts f32->bf16 inline
                nc.tensor.matmul(
                    it_ps[:],
                    a_sb[:, c, :],
                    xc[:],
                    start=(c == 0),
                    stop=(c == DC - 1),
                )

            # evacuate to bf16 (GEMM2 stationary operand)
            it_sb = it_pool.tile([R, TG], BF16)
            nc.vector.tensor_copy(it_sb[:], it_ps[:])

            for sub in range(TG // P):
                tt = t0 + sub * P
                bs = bs_pool.tile([P, D], F32)
                nc.sync.dma_start(bs[:], base[tt : tt + P, :])
                for o in range(OC):
                    y_ps = ps2.tile([P, OCH], F32)
                    nc.tensor.matmul(
                        y_ps[:],
                        it_sb[:, sub * P : (sub + 1) * P],
                        b_sb[:, o * OCH : (o + 1) * OCH],
                        start=True,
                        stop=True,
                    )
                    dst = bs[:, o * OCH : (o + 1) * OCH]
                    nc.any.tensor_add(dst, dst, y_ps[:])
                nc.scalar.dma_start(out[tt : tt + P, :], bs[:])


def shard_inputs(x, base_output, adapter_indices, lora_A, lora_B, lora_scaling):
    idx = np.asarray(adapter_indices).astype(np.int64)
    a_b = np.asarray(lora_A, dtype=np.float32)[idx]        # [B, D, R]
    b_b = np.asarray(lora_B, dtype=np.float32)[idx]        # [B, R, D]
    s_b = np.asarray(lora_scaling, dtype=np.float32)[idx]  # [B]
    b_scaled = (b_b * s_b[:, None, None]).astype(ml_dtypes.bfloat16)
    a_bf = a_b.astype(ml_dtypes.bfloat16)
    xs = np.asarray(x, dtype=np.float32)
    bs = np.asarray(base_output, dtype=np.float32)
    return [
        {
            "xt": np.ascontiguousarray(xs[b].T),  # [D, T]
            "base": np.ascontiguousarray(bs[b]),
            "a_w": np.ascontiguousarray(a_bf[b]),
            "b_w": np.ascontiguousarray(b_scaled[b]),
        }
        for b in range(B)
    ]


def run(inputs: dict, trace: bool = False):
    """Build + run on 8 cores. Returns (output [B,T,D] f32, BassKernelResults)."""
    from concourse.bass_utils import run_bass_kernel_spmd

    nc = build_program()
    in_maps = shard_inputs(**inputs)
    res = run_bass_kernel_spmd(nc, in_maps, core_ids=list(range(B)), trace=trace)
    out = np.stack([res.results[b]["out"] for b in range(B)], axis=0)
    return out, res


def kernel(x, base_output, adapter_indices, lora_A, lora_B, lora_scaling):
    out, _ = run(
        dict(
            x=x,
            base_output=base_output,
            adapter_indices=adapter_indices,
            lora_A=lora_A,
            lora_B=lora_B,
            lora_scaling=lora_scaling,
        )
    )
    return out
